# revision 19
# baseline (speedup 1.0000x reference)
"""Trainium2 Bass kernel for nn_CausalSelfAttention (tensor-parallel over heads, 8 cores).

Contract: kernel(**inputs) takes FULL unsharded numpy inputs and returns the
FULL output [1, 2048, 1024] float32. Internally: shards over 8 NeuronCores
(2 heads each, Wq/Wk/Wv column-sharded, Wo row-sharded), runs one SPMD Bass
program via run_bass_kernel_spmd, and sums the 8 partial Wo products on the
host (the row-parallel unshard).

Compute structure per core (heads 2c, 2c+1):
  - host passes x pre-transposed (xT [D, T]) and rotary tables with the
    s_eff scale folded in (rota/rotb [128, T], bf16)
  - projections in transposed layout: qT/kT/vT [128, T] = W.T @ xT
  - cosine-norm via squared-sums matmul + rsqrt; rotary as two elementwise
    muls + add with a partition-block swap done by SBUF-to-SBUF DMA
  - attention in S^T layout: S^T[ts, tq] = k̂T_chunk.T @ q̂T (row-packed two
    heads in the PE array), P^T = exp(0.12 S^T) on ACT (bf16 out), causal
    diagonal blocks masked by a precomputed 0/1 mask, softmax denominator
    accumulated on GpSimd and reduced with a ones-matmul, AV as
    V^T P^T col-packed per head into one PSUM bank
  - out rows = (yT/Z).T @ Wo_shard, partial [T, D] per core
Matmul operands are bf16 (TensorE runs fp32 at half rate); accumulation is
always fp32 in PSUM; softmax stats (sumsq, rsqrt, Z, 1/Z) kept in fp32.
"""

import os
import sys
import types

import numpy as np
import ml_dtypes

for _p in ("/opt/trn_rl_repo", "/root/.axon_site/_ro/trn_rl_repo"):
    if os.path.isdir(_p) and _p not in sys.path:
        sys.path.append(_p)

import concourse.bass as bass
import concourse.mybir as mybir
import concourse.tile as tile
from concourse.bass_utils import run_bass_kernel_spmd

F32 = mybir.dt.float32
BF16 = mybir.dt.bfloat16
NPBF16 = ml_dtypes.bfloat16
NCORES = 8
T = 2048
D = 1024
NH = 16
HD = 64
HPC = NH // NCORES   # heads per core
EPC = HPC * HD       # projection cols per core
ATTN_SCALE = 0.12
NT = T // 512
NK = D // 128

LAST = {}


def _register_ntff_hook():
    """Best-effort: register the axon NTFF profile hook if the image's antenv
    lacks axon_hooks (profiling only; compile/run work without it)."""
    try:
        import antenv.axon_hooks  # noqa: F401
        return
    except ImportError:
        pass
    try:
        import trn_agent_boot.trn_boot as tb

        mod = types.ModuleType("antenv.axon_hooks")
        holder = {}
        mod.set_axon_ntff_profile_hook = lambda h: holder.__setitem__("h", h)
        mod.get_axon_ntff_profile_hook = lambda: holder.get("h")
        sys.modules["antenv.axon_hooks"] = mod
        mod.set_axon_ntff_profile_hook(
            tb._ntff_profile_via_ctypes("/opt/axon/libaxon_pjrt.so")
        )
    except Exception:
        pass


def _split_ctrl_waits(nc, k_default=1):
    """The container's walrus build rejects instructions carrying more than one
    semaphore sync-wait; hoist extra waits onto single-wait NoOps that precede
    the instruction on the same engine queue (AND semantics preserved)."""
    n_nops = 0
    for f in nc.m.functions:
        for blk in f.blocks:
            new, changed = [], False
            for inst in list(blk.instructions):
                si = inst.sync_info
                waits = list(si.on_wait) if si is not None else []
                kmax = 1 if isinstance(inst, mybir.InstDrain) else k_default
                if len(waits) > kmax:
                    for k, w in enumerate(waits[:-kmax]):
                        nop = mybir.InstNoOp(name=f"{inst.name}-sw{k}", ins=[], outs=[])
                        nop.engine = inst.engine
                        nop.sync_info = mybir.SyncInfo(on_wait=[w], on_update=[])
                        new.append(nop)
                        n_nops += 1
                    inst.sync_info = mybir.SyncInfo(
                        on_wait=list(waits[-kmax:]), on_update=list(si.on_update)
                    )
                    changed = True
                new.append(inst)
            if changed:
                blk.instructions = new
    return n_nops


def _build_nc():
    nc = bass.Bass("TRN2", target_bir_lowering=False, debug=False, num_devices=NCORES)

    xT_d = nc.dram_tensor("xT", [D, T], BF16, kind="ExternalInput")
    wq_d = nc.dram_tensor("wq", [D, EPC], BF16, kind="ExternalInput")
    wk_d = nc.dram_tensor("wk", [D, EPC], BF16, kind="ExternalInput")
    wv_d = nc.dram_tensor("wv", [D, EPC], BF16, kind="ExternalInput")
    wo_d = nc.dram_tensor("wo", [EPC, D], BF16, kind="ExternalInput")
    rota_d = nc.dram_tensor("rota", [EPC, T], BF16, kind="ExternalInput")
    rotb_d = nc.dram_tensor("rotb", [EPC, T], BF16, kind="ExternalInput")
    hselw_d = nc.dram_tensor("hselw", [128, 128], BF16, kind="ExternalInput")
    out_d = nc.dram_tensor("out", [T, D], F32, kind="ExternalOutput")

    with tile.TileContext(nc) as tc:
        with (
            tc.tile_pool(name="wt", bufs=1) as wt,
            tc.tile_pool(name="big", bufs=9) as big,   # xT chunks + qrot/krot/vnat
            tc.tile_pool(name="praw", bufs=3) as prawp,
            tc.tile_pool(name="pn", bufs=2) as pnp,
            tc.tile_pool(name="psw", bufs=2) as pswp,
            tc.tile_pool(name="sm", bufs=2) as smp,
            tc.tile_pool(name="at", bufs=1) as atp,
            tc.tile_pool(name="mm", bufs=2, space="PSUM") as mmp,
            tc.tile_pool(name="ps", bufs=2, space="PSUM") as psp,
            tc.tile_pool(name="py", bufs=2, space="PSUM") as pyp,
        ):
            # ---- constants / weights ----
            wq_s = wt.tile([128, D], BF16, tag="wq")
            wk_s = wt.tile([128, D], BF16, tag="wk")
            wv_s = wt.tile([128, D], BF16, tag="wv")
            wo_s = wt.tile([EPC, D], BF16, tag="wo")
            rota = wt.tile([EPC, T], BF16, tag="rota")
            rotb = wt.tile([EPC, T], BF16, tag="rotb")
            hselw = wt.tile([128, 128], BF16, tag="hselw")
            eps = wt.tile([128, 1], F32, tag="eps")

            for w_s, w_d in ((wq_s, wq_d), (wk_s, wk_d), (wv_s, wv_d)):
                nc.sync.dma_start(
                    w_s[:].rearrange("p (i f) -> p i f", i=NK),
                    w_d[:].rearrange("(i p) f -> p i f", p=128),
                )
            nc.sync.dma_start(wo_s[:], wo_d[:])
            nc.sync.dma_start(rota[:], rota_d[:])
            nc.sync.dma_start(rotb[:], rotb_d[:])
            nc.sync.dma_start(hselw[:], hselw_d[:])
            nc.gpsimd.memset(eps[:], 1e-12)

            xc = []
            for i in range(NK):
                t_ = big.tile([128, T], BF16, tag="big")
                nc.sync.dma_start(t_[:], xT_d[128 * i : 128 * (i + 1), :])
                xc.append(t_)

            # ---- projections: transposed layout [EPC, T], bf16 out ----
            def project(w_s):
                raw = prawp.tile([EPC, T], BF16, tag="praw")
                for n in range(NT):
                    ps = mmp.tile([128, 512], F32, tag="mm")
                    for i in range(NK):
                        nc.tensor.matmul(
                            ps[:],
                            w_s[:, 128 * i : 128 * (i + 1)],
                            xc[i][:, 512 * n : 512 * (n + 1)],
                            start=(i == 0),
                            stop=(i == NK - 1),
                        )
                    nc.vector.tensor_copy(raw[:, 512 * n : 512 * (n + 1)], ps[:])
                return raw

            vT_raw = project(wv_s)
            q_raw = project(wq_s)
            k_raw = project(wk_s)

            # ---- cosine-norm + scale + rotary (per 512-chunk) ----
            # rsqrt(sumsq) = exp(-0.5 * ln(sumsq)): the sumsq matmul lhsT is
            # hselw[e,d] = (head(e)==head(d)) which both reduces and
            # broadcasts across each head's 64 rows in one shot. Ln and Exp
            # are batched (all Lns, then all Exps) so ACT loads each table
            # once per tensor instead of per chunk.
            def norm_rot_both(raws):
                rots = [
                    big.tile([EPC, T], BF16, name=f"rot_{ti}", tag="big")
                    for ti in range(2)
                ]
                lws, rws = {}, {}
                for ti, raw in enumerate(raws):
                    for n in range(NT):
                        sl = slice(512 * n, 512 * (n + 1))
                        sq = smp.tile([128, 512], BF16, name=f"sq_{ti}_{n}", tag="sq")
                        nc.vector.tensor_mul(sq[:], raw[:, sl], raw[:, sl])
                        ssb = mmp.tile([128, 512], F32, name=f"ssb_{ti}_{n}", tag="mm")
                        nc.tensor.matmul(ssb[:], hselw[:], sq[:], start=True, stop=True)
                        lw = smp.tile(
                            [128, 512], F32, name=f"lw_{ti}_{n}", tag="lw", bufs=8
                        )
                        nc.scalar.activation(
                            lw[:], ssb[:], mybir.ActivationFunctionType.Ln, bias=eps[:]
                        )
                        lws[(ti, n)] = lw
                for ti in range(2):
                    for n in range(NT):
                        rw = smp.tile(
                            [128, 512], BF16, name=f"rw_{ti}_{n}", tag="rw", bufs=8
                        )
                        nc.scalar.activation(
                            rw[:], lws[(ti, n)][:],
                            mybir.ActivationFunctionType.Exp, scale=-0.5,
                        )
                        rws[(ti, n)] = rw
                # chunk-major so attention's first tq chunk unblocks earliest
                for n in range(NT):
                    for ti, raw in enumerate(raws):
                        sl = slice(512 * n, 512 * (n + 1))
                        qn = pnp.tile([128, 512], BF16, name=f"qn_{ti}_{n}", tag="pn")
                        nc.vector.tensor_mul(qn[:], raw[:, sl], rws[(ti, n)][:])
                        sw = pswp.tile([128, 512], BF16, name=f"sw_{ti}_{n}", tag="psw")
                        for (a, b) in ((0, 32), (32, 0), (64, 96), (96, 64)):
                            nc.gpsimd.dma_start(sw[a : a + 32, :], qn[b : b + 32, :])
                        nc.vector.tensor_mul(rots[ti][:, sl], qn[:], rota[:, sl])
                        nc.vector.tensor_mul(sw[:], sw[:], rotb[:, sl])
                        nc.vector.tensor_add(rots[ti][:, sl], rots[ti][:, sl], sw[:])
                return rots

            # ---- v to natural layout via hardware DMA transpose (bf16) ----
            # per 128-chunk j the layout is [v_h0 (64) | ones (64) | v_h1 (64)
            # | ones (64)], so each AV matmul lhsT is [128, 128] and writes
            # y_h to PSUM rows 0-63 and the softmax denominator Z_h
            # (pre-broadcast 64x) to rows 64-127.
            vext = wt.tile([128, (T // 128) * 256], BF16, tag="vext")
            nc.gpsimd.memset(vext[:], 1.0)
            ident = wt.tile([128, 128], BF16, tag="ident")
            nc.gpsimd.memset(ident[:], 0.0)
            nc.gpsimd.affine_select(
                out=ident[:],
                in_=ident[:],
                compare_op=mybir.AluOpType.not_equal,
                fill=1.0,
                base=0,
                pattern=[[-1, 128]],
                channel_multiplier=1,
            )
            vv = vext[:].rearrange("p (j h f) -> p j h f", h=HPC, f=128)
            for j in range(T // 128):
                tp_ = psp.tile([128, 128], BF16, tag="ps", name=f"vtp_{j}")
                nc.tensor.transpose(
                    tp_[:], vT_raw[:, 128 * j : 128 * (j + 1)], ident[:]
                )
                # rows->[ts], cols [h0 d | h1 d]; scatter the two head halves
                # into the [v | ones] slots of vext with one strided copy
                nc.vector.tensor_copy(
                    vv[:, j, :, 0:64],
                    tp_[:].rearrange("p (h f) -> p h f", h=HPC),
                )


            qrot, krot = norm_rot_both([q_raw, k_raw])

            # ---- attention (S^T layout) + Wo partial ----
            for c in range(NT):
                nts = 4 * c + 4
                cq = slice(512 * c, 512 * (c + 1))
                pyh = [
                    pyp.tile([128, 512], F32, name=f"py0_{c}", tag="py0", bufs=2),
                    pyp.tile([128, 512], F32, name=f"py1_{c}", tag="py1", bufs=2),
                ]
                for j in range(nts):
                    m = j - 4 * c
                    for h in range(HPC):
                        hs = slice(64 * h, 64 * (h + 1))
                        ps = psp.tile([128, 512], F32, tag="ps")
                        nc.tensor.matmul(
                            ps[:],
                            krot[hs, 128 * j : 128 * (j + 1)],
                            qrot[hs, cq],
                            start=True,
                            stop=True,
                            tile_position=(64 * h, 0),
                        )
                        pt = atp.tile([128, 512], BF16, tag=f"pt{h}", bufs=3)
                        nc.scalar.activation(
                            pt[:], ps[:], mybir.ActivationFunctionType.Exp,
                            scale=ATTN_SCALE,
                        )
                        if m >= 0:
                            # causal: keep pt[x, y] only where y >= x + 128*m
                            nc.gpsimd.affine_select(
                                out=pt[:],
                                in_=pt[:],
                                compare_op=mybir.AluOpType.is_ge,
                                fill=0.0,
                                base=-128 * m,
                                pattern=[[1, 512]],
                                channel_multiplier=-1,
                            )
                        nc.tensor.matmul(
                            pyh[h][:],
                            vext[:, 256 * j + 128 * h : 256 * j + 128 * (h + 1)],
                            pt[:],
                            start=(j == 0),
                            stop=(j == nts - 1),
                        )
                # normalize: yt rows of head h = py_h[0:64] * (1/Z_h);
                # Z_h sits pre-broadcast in py_h rows 64-127
                yt = atp.tile([128, 512], BF16, tag="yt", bufs=2)
                for h in range(HPC):
                    hs = slice(64 * h, 64 * (h + 1))
                    zs = smp.tile([64, 512], F32, tag="zs")
                    nc.vector.tensor_copy(zs[:], pyh[h][64:128, :])
                    zri = smp.tile([64, 512], F32, tag="zri")
                    nc.vector.reciprocal(zri[:], zs[:])
                    nc.vector.tensor_mul(yt[hs, :], pyh[h][0:64, :], zri[:])
                # Wo partial: out[tq, :] = yt[:, tq].T @ wo
                for mi in range(4):
                    ost = atp.tile([128, D], F32, tag="ost", bufs=2)
                    for nn in range(2):
                        po = mmp.tile([128, 512], F32, tag="mm")
                        nc.tensor.matmul(
                            po[:],
                            yt[:, 128 * mi : 128 * (mi + 1)],
                            wo_s[:, 512 * nn : 512 * (nn + 1)],
                            start=True,
                            stop=True,
                        )
                        if nn == 0:
                            nc.vector.tensor_copy(ost[:, 512 * nn : 512 * (nn + 1)], po[:])
                        else:
                            nc.scalar.copy(ost[:, 512 * nn : 512 * (nn + 1)], po[:])
                    r0 = 512 * c + 128 * mi
                    nc.sync.dma_start(out_d[r0 : r0 + 128, :], ost[:])

    return nc


_NC = None
_NC_SPLIT = False


def _host_shards(x, Wq, Wk, Wv, Wo, s_qk):
    x = np.asarray(x, dtype=np.float32)
    Wq = np.asarray(Wq, dtype=np.float32)
    Wk = np.asarray(Wk, dtype=np.float32)
    Wv = np.asarray(Wv, dtype=np.float32)
    Wo = np.asarray(Wo, dtype=np.float32)
    s_qk = np.asarray(s_qk, dtype=np.float32)

    xT = np.ascontiguousarray(x.reshape(T, D).T).astype(NPBF16)

    dim_q = HD // 4
    freq = (1.0 / 1024.0) ** np.linspace(0.0, 1.0, dim_q, dtype=np.float32)
    freq = np.concatenate([freq, np.zeros(dim_q, np.float32)])
    theta = np.arange(T, dtype=np.float32)[:, None] * freq[None, :]
    cosT = np.cos(theta).T.astype(np.float32)
    sinT = np.sin(theta).T.astype(np.float32)
    A64 = np.concatenate([cosT, cosT], 0)
    B64 = np.concatenate([sinT, -sinT], 0)
    s_eff = s_qk * np.float32(np.sqrt(D))

    hselw = np.zeros((128, 128), np.float32)
    for h in range(HPC):
        hselw[64 * h : 64 * (h + 1), 64 * h : 64 * (h + 1)] = 1.0
    hselw = hselw.astype(NPBF16)

    in_maps = []
    for c in range(NCORES):
        cols = slice(EPC * c, EPC * (c + 1))
        rota_rows, rotb_rows = [], []
        for h in range(HPC):
            s = s_eff[HPC * c + h]
            s_swap = np.concatenate([s[32:], s[:32]])
            rota_rows.append(s[:, None] * A64)
            rotb_rows.append(s_swap[:, None] * B64)
        in_maps.append(
            {
                "xT": xT,
                "wq": np.ascontiguousarray(Wq[:, cols]).astype(NPBF16),
                "wk": np.ascontiguousarray(Wk[:, cols]).astype(NPBF16),
                "wv": np.ascontiguousarray(Wv[:, cols]).astype(NPBF16),
                "wo": np.ascontiguousarray(Wo[EPC * c : EPC * (c + 1), :]).astype(NPBF16),
                "rota": np.concatenate(rota_rows, 0).astype(NPBF16),
                "rotb": np.concatenate(rotb_rows, 0).astype(NPBF16),
                "hselw": hselw,
            }
        )
    return in_maps


def kernel(x, Wq, Wk, Wv, Wo, s_qk):
    global _NC, _NC_SPLIT
    _register_ntff_hook()
    if _NC is None:
        _NC = _build_nc()
    if not _NC_SPLIT:
        _split_ctrl_waits(_NC)
        _NC_SPLIT = True
    in_maps = _host_shards(x, Wq, Wk, Wv, Wo, s_qk)
    res = None
    last_exc = None
    for _attempt in range(3):
        try:
            res = run_bass_kernel_spmd(_NC, in_maps, list(range(NCORES)))
            break
        except Exception as e:  # transient NRT_EXEC_UNIT_UNRECOVERABLE seen on first runs
            last_exc = e
    if res is None:
        raise last_exc
    LAST["exec_time_ns"] = res.exec_time_ns
    LAST["trace"] = res.instructions_and_trace[1] if res.instructions_and_trace else None
    total = np.zeros((T, D), np.float64)
    for i in range(NCORES):
        total += res.results[i]["out"].astype(np.float64)
    return total.astype(np.float32).reshape(1, T, D)


# revision 20
# speedup vs baseline: 1.0043x; 1.0043x over previous
"""Trainium2 Bass kernel for nn_CausalSelfAttention (tensor-parallel over heads, 8 cores).

Contract: kernel(**inputs) takes FULL unsharded numpy inputs and returns the
FULL output [1, 2048, 1024] float32. Internally: shards over 8 NeuronCores
(2 heads each, Wq/Wk/Wv column-sharded, Wo row-sharded), runs one SPMD Bass
program via run_bass_kernel_spmd, and sums the 8 partial Wo products on the
host (the row-parallel unshard).

Compute structure per core (heads 2c, 2c+1):
  - host passes x pre-transposed (xT [D, T]) and rotary tables with the
    s_eff scale folded in (rota/rotb [128, T], bf16)
  - projections in transposed layout: qT/kT/vT [128, T] = W.T @ xT
  - cosine-norm via squared-sums matmul + rsqrt; rotary as two elementwise
    muls + add with a partition-block swap done by SBUF-to-SBUF DMA
  - attention in S^T layout: S^T[ts, tq] = k̂T_chunk.T @ q̂T (row-packed two
    heads in the PE array), P^T = exp(0.12 S^T) on ACT (bf16 out), causal
    diagonal blocks masked by a precomputed 0/1 mask, softmax denominator
    accumulated on GpSimd and reduced with a ones-matmul, AV as
    V^T P^T col-packed per head into one PSUM bank
  - out rows = (yT/Z).T @ Wo_shard, partial [T, D] per core
Matmul operands are bf16 (TensorE runs fp32 at half rate); accumulation is
always fp32 in PSUM; softmax stats (sumsq, rsqrt, Z, 1/Z) kept in fp32.
"""

import os
import sys
import types

import numpy as np
import ml_dtypes

for _p in ("/opt/trn_rl_repo", "/root/.axon_site/_ro/trn_rl_repo"):
    if os.path.isdir(_p) and _p not in sys.path:
        sys.path.append(_p)

import concourse.bass as bass
import concourse.mybir as mybir
import concourse.tile as tile
from concourse.bass_utils import run_bass_kernel_spmd

F32 = mybir.dt.float32
BF16 = mybir.dt.bfloat16
NPBF16 = ml_dtypes.bfloat16
NCORES = 8
T = 2048
D = 1024
NH = 16
HD = 64
HPC = NH // NCORES   # heads per core
EPC = HPC * HD       # projection cols per core
ATTN_SCALE = 0.12
NT = T // 512
NK = D // 128

LAST = {}


def _register_ntff_hook():
    """Best-effort: register the axon NTFF profile hook if the image's antenv
    lacks axon_hooks (profiling only; compile/run work without it)."""
    try:
        import antenv.axon_hooks  # noqa: F401
        return
    except ImportError:
        pass
    try:
        import trn_agent_boot.trn_boot as tb

        mod = types.ModuleType("antenv.axon_hooks")
        holder = {}
        mod.set_axon_ntff_profile_hook = lambda h: holder.__setitem__("h", h)
        mod.get_axon_ntff_profile_hook = lambda: holder.get("h")
        sys.modules["antenv.axon_hooks"] = mod
        mod.set_axon_ntff_profile_hook(
            tb._ntff_profile_via_ctypes("/opt/axon/libaxon_pjrt.so")
        )
    except Exception:
        pass


def _split_ctrl_waits(nc, k_default=1):
    """The container's walrus build rejects instructions carrying more than one
    semaphore sync-wait; hoist extra waits onto single-wait NoOps that precede
    the instruction on the same engine queue (AND semantics preserved)."""
    n_nops = 0
    for f in nc.m.functions:
        for blk in f.blocks:
            new, changed = [], False
            for inst in list(blk.instructions):
                si = inst.sync_info
                waits = list(si.on_wait) if si is not None else []
                kmax = 1 if isinstance(inst, mybir.InstDrain) else k_default
                if len(waits) > kmax:
                    for k, w in enumerate(waits[:-kmax]):
                        nop = mybir.InstNoOp(name=f"{inst.name}-sw{k}", ins=[], outs=[])
                        nop.engine = inst.engine
                        nop.sync_info = mybir.SyncInfo(on_wait=[w], on_update=[])
                        new.append(nop)
                        n_nops += 1
                    inst.sync_info = mybir.SyncInfo(
                        on_wait=list(waits[-kmax:]), on_update=list(si.on_update)
                    )
                    changed = True
                new.append(inst)
            if changed:
                blk.instructions = new
    return n_nops


def _build_nc():
    nc = bass.Bass("TRN2", target_bir_lowering=False, debug=False, num_devices=NCORES)

    xT_d = nc.dram_tensor("xT", [D, T], BF16, kind="ExternalInput")
    wq_d = nc.dram_tensor("wq", [D, EPC], BF16, kind="ExternalInput")
    wk_d = nc.dram_tensor("wk", [D, EPC], BF16, kind="ExternalInput")
    wv_d = nc.dram_tensor("wv", [D, EPC], BF16, kind="ExternalInput")
    wo_d = nc.dram_tensor("wo", [EPC, D], BF16, kind="ExternalInput")
    rota_d = nc.dram_tensor("rota", [EPC, T], BF16, kind="ExternalInput")
    rotb_d = nc.dram_tensor("rotb", [EPC, T], BF16, kind="ExternalInput")
    hselw_d = nc.dram_tensor("hselw", [128, 128], BF16, kind="ExternalInput")
    out_d = nc.dram_tensor("out", [T, D], F32, kind="ExternalOutput")

    with tile.TileContext(nc) as tc:
        with (
            tc.tile_pool(name="wt", bufs=1) as wt,
            tc.tile_pool(name="big", bufs=9) as big,   # xT chunks + qrot/krot/vnat
            tc.tile_pool(name="praw", bufs=3) as prawp,
            tc.tile_pool(name="pn", bufs=2) as pnp,
            tc.tile_pool(name="psw", bufs=2) as pswp,
            tc.tile_pool(name="sm", bufs=2) as smp,
            tc.tile_pool(name="at", bufs=1) as atp,
            tc.tile_pool(name="mm", bufs=2, space="PSUM") as mmp,
            tc.tile_pool(name="ps", bufs=4, space="PSUM") as psp,
            tc.tile_pool(name="py", bufs=2, space="PSUM") as pyp,
        ):
            # ---- constants / weights ----
            wq_s = wt.tile([128, D], BF16, tag="wq")
            wk_s = wt.tile([128, D], BF16, tag="wk")
            wv_s = wt.tile([128, D], BF16, tag="wv")
            wo_s = wt.tile([EPC, D], BF16, tag="wo")
            rota = wt.tile([EPC, T], BF16, tag="rota")
            rotb = wt.tile([EPC, T], BF16, tag="rotb")
            hselw = wt.tile([128, 128], BF16, tag="hselw")
            eps = wt.tile([128, 1], F32, tag="eps")

            for w_s, w_d in ((wq_s, wq_d), (wk_s, wk_d), (wv_s, wv_d)):
                nc.sync.dma_start(
                    w_s[:].rearrange("p (i f) -> p i f", i=NK),
                    w_d[:].rearrange("(i p) f -> p i f", p=128),
                )
            nc.sync.dma_start(wo_s[:], wo_d[:])
            nc.sync.dma_start(rota[:], rota_d[:])
            nc.sync.dma_start(rotb[:], rotb_d[:])
            nc.sync.dma_start(hselw[:], hselw_d[:])
            nc.gpsimd.memset(eps[:], 1e-12)

            xc = []
            for i in range(NK):
                t_ = big.tile([128, T], BF16, tag="big")
                nc.sync.dma_start(t_[:], xT_d[128 * i : 128 * (i + 1), :])
                xc.append(t_)

            # ---- projections: transposed layout [EPC, T], bf16 out ----
            def project(w_s):
                raw = prawp.tile([EPC, T], BF16, tag="praw")
                for n in range(NT):
                    ps = mmp.tile([128, 512], F32, tag="mm")
                    for i in range(NK):
                        nc.tensor.matmul(
                            ps[:],
                            w_s[:, 128 * i : 128 * (i + 1)],
                            xc[i][:, 512 * n : 512 * (n + 1)],
                            start=(i == 0),
                            stop=(i == NK - 1),
                        )
                    nc.vector.tensor_copy(raw[:, 512 * n : 512 * (n + 1)], ps[:])
                return raw

            vT_raw = project(wv_s)
            q_raw = project(wq_s)
            k_raw = project(wk_s)

            # ---- cosine-norm + scale + rotary (per 512-chunk) ----
            # rsqrt(sumsq) = exp(-0.5 * ln(sumsq)): the sumsq matmul lhsT is
            # hselw[e,d] = (head(e)==head(d)) which both reduces and
            # broadcasts across each head's 64 rows in one shot. Ln and Exp
            # are batched (all Lns, then all Exps) so ACT loads each table
            # once per tensor instead of per chunk.
            def norm_rot_both(raws):
                rots = [
                    big.tile([EPC, T], BF16, name=f"rot_{ti}", tag="big")
                    for ti in range(2)
                ]
                lws, rws = {}, {}
                for ti, raw in enumerate(raws):
                    for n in range(NT):
                        sl = slice(512 * n, 512 * (n + 1))
                        sq = smp.tile([128, 512], BF16, name=f"sq_{ti}_{n}", tag="sq")
                        nc.vector.tensor_mul(sq[:], raw[:, sl], raw[:, sl])
                        ssb = mmp.tile([128, 512], F32, name=f"ssb_{ti}_{n}", tag="mm")
                        nc.tensor.matmul(ssb[:], hselw[:], sq[:], start=True, stop=True)
                        lw = smp.tile(
                            [128, 512], F32, name=f"lw_{ti}_{n}", tag="lw", bufs=8
                        )
                        nc.scalar.activation(
                            lw[:], ssb[:], mybir.ActivationFunctionType.Ln, bias=eps[:]
                        )
                        lws[(ti, n)] = lw
                for ti in range(2):
                    for n in range(NT):
                        rw = smp.tile(
                            [128, 512], BF16, name=f"rw_{ti}_{n}", tag="rw", bufs=8
                        )
                        nc.scalar.activation(
                            rw[:], lws[(ti, n)][:],
                            mybir.ActivationFunctionType.Exp, scale=-0.5,
                        )
                        rws[(ti, n)] = rw
                # chunk-major so attention's first tq chunk unblocks earliest
                for n in range(NT):
                    for ti, raw in enumerate(raws):
                        sl = slice(512 * n, 512 * (n + 1))
                        qn = pnp.tile([128, 512], BF16, name=f"qn_{ti}_{n}", tag="pn")
                        nc.vector.tensor_mul(qn[:], raw[:, sl], rws[(ti, n)][:])
                        sw = pswp.tile([128, 512], BF16, name=f"sw_{ti}_{n}", tag="psw")
                        for (a, b) in ((0, 32), (32, 0), (64, 96), (96, 64)):
                            nc.gpsimd.dma_start(sw[a : a + 32, :], qn[b : b + 32, :])
                        nc.vector.tensor_mul(rots[ti][:, sl], qn[:], rota[:, sl])
                        nc.vector.tensor_mul(sw[:], sw[:], rotb[:, sl])
                        nc.vector.tensor_add(rots[ti][:, sl], rots[ti][:, sl], sw[:])
                return rots

            # ---- v to natural layout via hardware DMA transpose (bf16) ----
            # per 128-chunk j the layout is [v_h0 (64) | ones (64) | v_h1 (64)
            # | ones (64)], so each AV matmul lhsT is [128, 128] and writes
            # y_h to PSUM rows 0-63 and the softmax denominator Z_h
            # (pre-broadcast 64x) to rows 64-127.
            vext = wt.tile([128, (T // 128) * 256], BF16, tag="vext")
            nc.gpsimd.memset(vext[:], 1.0)
            ident = wt.tile([128, 128], BF16, tag="ident")
            nc.gpsimd.memset(ident[:], 0.0)
            nc.gpsimd.affine_select(
                out=ident[:],
                in_=ident[:],
                compare_op=mybir.AluOpType.not_equal,
                fill=1.0,
                base=0,
                pattern=[[-1, 128]],
                channel_multiplier=1,
            )
            vv = vext[:].rearrange("p (j h f) -> p j h f", h=HPC, f=128)
            for j in range(T // 128):
                tp_ = psp.tile([128, 128], BF16, tag="ps", name=f"vtp_{j}")
                nc.tensor.transpose(
                    tp_[:], vT_raw[:, 128 * j : 128 * (j + 1)], ident[:]
                )
                # rows->[ts], cols [h0 d | h1 d]; scatter the two head halves
                # into the [v | ones] slots of vext with one strided copy
                nc.vector.tensor_copy(
                    vv[:, j, :, 0:64],
                    tp_[:].rearrange("p (h f) -> p h f", h=HPC),
                )


            qrot, krot = norm_rot_both([q_raw, k_raw])

            # ---- attention (S^T layout) + Wo partial ----
            for c in range(NT):
                nts = 4 * c + 4
                cq = slice(512 * c, 512 * (c + 1))
                pyh = [
                    pyp.tile([128, 512], F32, name=f"py0_{c}", tag="py0", bufs=1),
                    pyp.tile([128, 512], F32, name=f"py1_{c}", tag="py1", bufs=1),
                ]
                for j in range(nts):
                    m = j - 4 * c
                    for h in range(HPC):
                        hs = slice(64 * h, 64 * (h + 1))
                        ps = psp.tile([128, 512], F32, tag="ps")
                        nc.tensor.matmul(
                            ps[:],
                            krot[hs, 128 * j : 128 * (j + 1)],
                            qrot[hs, cq],
                            start=True,
                            stop=True,
                            tile_position=(64 * h, 0),
                        )
                        pt = atp.tile([128, 512], BF16, tag=f"pt{h}", bufs=3)
                        nc.scalar.activation(
                            pt[:], ps[:], mybir.ActivationFunctionType.Exp,
                            scale=ATTN_SCALE,
                        )
                        if m >= 0:
                            # causal: keep pt[x, y] only where y >= x + 128*m
                            nc.gpsimd.affine_select(
                                out=pt[:],
                                in_=pt[:],
                                compare_op=mybir.AluOpType.is_ge,
                                fill=0.0,
                                base=-128 * m,
                                pattern=[[1, 512]],
                                channel_multiplier=-1,
                            )
                        nc.tensor.matmul(
                            pyh[h][:],
                            vext[:, 256 * j + 128 * h : 256 * j + 128 * (h + 1)],
                            pt[:],
                            start=(j == 0),
                            stop=(j == nts - 1),
                        )
                # normalize: yt rows of head h = py_h[0:64] * (1/Z_h);
                # Z_h sits pre-broadcast in py_h rows 64-127
                # copy y and Z out of PSUM immediately so the banks free for
                # the next chunk's AV; the slow reciprocal then runs on SBUF
                yt = atp.tile([128, 512], BF16, tag="yt", bufs=2)
                for h in range(HPC):
                    hs = slice(64 * h, 64 * (h + 1))
                    ysb = smp.tile([64, 512], F32, name=f"ysb_{c}_{h}", tag="ysb")
                    nc.vector.tensor_copy(ysb[:], pyh[h][0:64, :])
                    zs = smp.tile([64, 512], F32, name=f"zs_{c}_{h}", tag="zs")
                    nc.vector.tensor_copy(zs[:], pyh[h][64:128, :])
                    zri = smp.tile([64, 512], F32, name=f"zri_{c}_{h}", tag="zri")
                    nc.vector.reciprocal(zri[:], zs[:])
                    nc.vector.tensor_mul(yt[hs, :], ysb[:], zri[:])
                # Wo partial: out[tq, :] = yt[:, tq].T @ wo
                for mi in range(4):
                    ost = atp.tile([128, D], F32, tag="ost", bufs=2)
                    for nn in range(2):
                        po = mmp.tile([128, 512], F32, tag="mm")
                        nc.tensor.matmul(
                            po[:],
                            yt[:, 128 * mi : 128 * (mi + 1)],
                            wo_s[:, 512 * nn : 512 * (nn + 1)],
                            start=True,
                            stop=True,
                        )
                        if nn == 0:
                            nc.vector.tensor_copy(ost[:, 512 * nn : 512 * (nn + 1)], po[:])
                        else:
                            nc.scalar.copy(ost[:, 512 * nn : 512 * (nn + 1)], po[:])
                    r0 = 512 * c + 128 * mi
                    nc.sync.dma_start(out_d[r0 : r0 + 128, :], ost[:])

    return nc


_NC = None
_NC_SPLIT = False


def _host_shards(x, Wq, Wk, Wv, Wo, s_qk):
    x = np.asarray(x, dtype=np.float32)
    Wq = np.asarray(Wq, dtype=np.float32)
    Wk = np.asarray(Wk, dtype=np.float32)
    Wv = np.asarray(Wv, dtype=np.float32)
    Wo = np.asarray(Wo, dtype=np.float32)
    s_qk = np.asarray(s_qk, dtype=np.float32)

    xT = np.ascontiguousarray(x.reshape(T, D).T).astype(NPBF16)

    dim_q = HD // 4
    freq = (1.0 / 1024.0) ** np.linspace(0.0, 1.0, dim_q, dtype=np.float32)
    freq = np.concatenate([freq, np.zeros(dim_q, np.float32)])
    theta = np.arange(T, dtype=np.float32)[:, None] * freq[None, :]
    cosT = np.cos(theta).T.astype(np.float32)
    sinT = np.sin(theta).T.astype(np.float32)
    A64 = np.concatenate([cosT, cosT], 0)
    B64 = np.concatenate([sinT, -sinT], 0)
    s_eff = s_qk * np.float32(np.sqrt(D))

    hselw = np.zeros((128, 128), np.float32)
    for h in range(HPC):
        hselw[64 * h : 64 * (h + 1), 64 * h : 64 * (h + 1)] = 1.0
    hselw = hselw.astype(NPBF16)

    in_maps = []
    for c in range(NCORES):
        cols = slice(EPC * c, EPC * (c + 1))
        rota_rows, rotb_rows = [], []
        for h in range(HPC):
            s = s_eff[HPC * c + h]
            s_swap = np.concatenate([s[32:], s[:32]])
            rota_rows.append(s[:, None] * A64)
            rotb_rows.append(s_swap[:, None] * B64)
        in_maps.append(
            {
                "xT": xT,
                "wq": np.ascontiguousarray(Wq[:, cols]).astype(NPBF16),
                "wk": np.ascontiguousarray(Wk[:, cols]).astype(NPBF16),
                "wv": np.ascontiguousarray(Wv[:, cols]).astype(NPBF16),
                "wo": np.ascontiguousarray(Wo[EPC * c : EPC * (c + 1), :]).astype(NPBF16),
                "rota": np.concatenate(rota_rows, 0).astype(NPBF16),
                "rotb": np.concatenate(rotb_rows, 0).astype(NPBF16),
                "hselw": hselw,
            }
        )
    return in_maps


def kernel(x, Wq, Wk, Wv, Wo, s_qk):
    global _NC, _NC_SPLIT
    _register_ntff_hook()
    if _NC is None:
        _NC = _build_nc()
    if not _NC_SPLIT:
        _split_ctrl_waits(_NC)
        _NC_SPLIT = True
    in_maps = _host_shards(x, Wq, Wk, Wv, Wo, s_qk)
    res = None
    last_exc = None
    for _attempt in range(3):
        try:
            res = run_bass_kernel_spmd(_NC, in_maps, list(range(NCORES)))
            break
        except Exception as e:  # transient NRT_EXEC_UNIT_UNRECOVERABLE seen on first runs
            last_exc = e
    if res is None:
        raise last_exc
    LAST["exec_time_ns"] = res.exec_time_ns
    LAST["trace"] = res.instructions_and_trace[1] if res.instructions_and_trace else None
    total = np.zeros((T, D), np.float64)
    for i in range(NCORES):
        total += res.results[i]["out"].astype(np.float64)
    return total.astype(np.float32).reshape(1, T, D)


# revision 21
# speedup vs baseline: 1.1508x; 1.1459x over previous
"""Trainium2 Bass kernel for nn_CausalSelfAttention (tensor-parallel over heads, 8 cores).

Contract: kernel(**inputs) takes FULL unsharded numpy inputs and returns the
FULL output [1, 2048, 1024] float32. Internally: shards over 8 NeuronCores
(2 heads each, Wq/Wk/Wv column-sharded, Wo row-sharded), runs one SPMD Bass
program via run_bass_kernel_spmd, and sums the 8 partial Wo products on the
host (the row-parallel unshard).

Compute structure per core (heads 2c, 2c+1):
  - host passes x pre-transposed (xT [D, T]) and rotary tables with the
    s_eff scale folded in (rota/rotb [128, T], bf16)
  - projections in transposed layout: qT/kT/vT [128, T] = W.T @ xT
  - cosine-norm via squared-sums matmul + rsqrt; rotary as two elementwise
    muls + add with a partition-block swap done by SBUF-to-SBUF DMA
  - attention in S^T layout: S^T[ts, tq] = k̂T_chunk.T @ q̂T (row-packed two
    heads in the PE array), P^T = exp(0.12 S^T) on ACT (bf16 out), causal
    diagonal blocks masked by a precomputed 0/1 mask, softmax denominator
    accumulated on GpSimd and reduced with a ones-matmul, AV as
    V^T P^T col-packed per head into one PSUM bank
  - out rows = (yT/Z).T @ Wo_shard, partial [T, D] per core
Matmul operands are bf16 (TensorE runs fp32 at half rate); accumulation is
always fp32 in PSUM; softmax stats (sumsq, rsqrt, Z, 1/Z) kept in fp32.
"""

import os
import sys
import types

import numpy as np
import ml_dtypes

for _p in ("/opt/trn_rl_repo", "/root/.axon_site/_ro/trn_rl_repo"):
    if os.path.isdir(_p) and _p not in sys.path:
        sys.path.append(_p)

import concourse.bass as bass
import concourse.mybir as mybir
import concourse.tile as tile
from concourse.bass_utils import run_bass_kernel_spmd

F32 = mybir.dt.float32
BF16 = mybir.dt.bfloat16
NPBF16 = ml_dtypes.bfloat16
NCORES = 8
T = 2048
D = 1024
NH = 16
HD = 64
HPC = NH // NCORES   # heads per core
EPC = HPC * HD       # projection cols per core
ATTN_SCALE = 0.12
NT = T // 512
NK = D // 128

LAST = {}


def _register_ntff_hook():
    """Best-effort: register the axon NTFF profile hook if the image's antenv
    lacks axon_hooks (profiling only; compile/run work without it)."""
    try:
        import antenv.axon_hooks  # noqa: F401
        return
    except ImportError:
        pass
    try:
        import trn_agent_boot.trn_boot as tb

        mod = types.ModuleType("antenv.axon_hooks")
        holder = {}
        mod.set_axon_ntff_profile_hook = lambda h: holder.__setitem__("h", h)
        mod.get_axon_ntff_profile_hook = lambda: holder.get("h")
        sys.modules["antenv.axon_hooks"] = mod
        mod.set_axon_ntff_profile_hook(
            tb._ntff_profile_via_ctypes("/opt/axon/libaxon_pjrt.so")
        )
    except Exception:
        pass


def _split_ctrl_waits(nc, k_default=1):
    """The container's walrus build rejects instructions carrying more than one
    semaphore sync-wait; hoist extra waits onto single-wait NoOps that precede
    the instruction on the same engine queue (AND semantics preserved)."""
    n_nops = 0
    for f in nc.m.functions:
        for blk in f.blocks:
            new, changed = [], False
            for inst in list(blk.instructions):
                si = inst.sync_info
                waits = list(si.on_wait) if si is not None else []
                kmax = 1 if isinstance(inst, mybir.InstDrain) else k_default
                if len(waits) > kmax:
                    for k, w in enumerate(waits[:-kmax]):
                        nop = mybir.InstNoOp(name=f"{inst.name}-sw{k}", ins=[], outs=[])
                        nop.engine = inst.engine
                        nop.sync_info = mybir.SyncInfo(on_wait=[w], on_update=[])
                        new.append(nop)
                        n_nops += 1
                    inst.sync_info = mybir.SyncInfo(
                        on_wait=list(waits[-kmax:]), on_update=list(si.on_update)
                    )
                    changed = True
                new.append(inst)
            if changed:
                blk.instructions = new
    return n_nops


def _build_nc():
    nc = bass.Bass("TRN2", target_bir_lowering=False, debug=False, num_devices=NCORES)

    xT_d = nc.dram_tensor("xT", [D, T], BF16, kind="ExternalInput")
    wq_d = nc.dram_tensor("wq", [D, EPC], BF16, kind="ExternalInput")
    wk_d = nc.dram_tensor("wk", [D, EPC], BF16, kind="ExternalInput")
    wv_d = nc.dram_tensor("wv", [D, EPC], BF16, kind="ExternalInput")
    wo_d = nc.dram_tensor("wo", [EPC, D], BF16, kind="ExternalInput")
    rota_d = nc.dram_tensor("rota", [EPC, T], BF16, kind="ExternalInput")
    rotb_d = nc.dram_tensor("rotb", [EPC, T], BF16, kind="ExternalInput")
    hselw_d = nc.dram_tensor("hselw", [128, 128], BF16, kind="ExternalInput")
    out_d = nc.dram_tensor("out", [T, D], F32, kind="ExternalOutput")

    with tile.TileContext(nc) as tc:
        with (
            tc.tile_pool(name="wt", bufs=1) as wt,
            tc.tile_pool(name="big", bufs=9) as big,   # xT chunks + qrot/krot/vnat
            tc.tile_pool(name="praw", bufs=3) as prawp,
            tc.tile_pool(name="pn", bufs=2) as pnp,
            tc.tile_pool(name="psw", bufs=2) as pswp,
            tc.tile_pool(name="sm", bufs=2) as smp,
            tc.tile_pool(name="at", bufs=1) as atp,
            tc.tile_pool(name="mm", bufs=2, space="PSUM") as mmp,
            tc.tile_pool(name="ps", bufs=4, space="PSUM") as psp,
            tc.tile_pool(name="py", bufs=2, space="PSUM") as pyp,
        ):
            # ---- constants / weights ----
            wq_s = wt.tile([128, D], BF16, tag="wq")
            wk_s = wt.tile([128, D], BF16, tag="wk")
            wv_s = wt.tile([128, D], BF16, tag="wv")
            wo_s = wt.tile([EPC, D], BF16, tag="wo")
            rota = wt.tile([EPC, T], BF16, tag="rota")
            rotb = wt.tile([EPC, T], BF16, tag="rotb")
            hselw = wt.tile([128, 128], BF16, tag="hselw")
            eps = wt.tile([128, 1], F32, tag="eps")

            for w_s, w_d in ((wq_s, wq_d), (wk_s, wk_d), (wv_s, wv_d)):
                nc.sync.dma_start(
                    w_s[:].rearrange("p (i f) -> p i f", i=NK),
                    w_d[:].rearrange("(i p) f -> p i f", p=128),
                )
            nc.sync.dma_start(wo_s[:], wo_d[:])
            nc.sync.dma_start(rota[:], rota_d[:])
            nc.sync.dma_start(rotb[:], rotb_d[:])
            nc.sync.dma_start(hselw[:], hselw_d[:])
            nc.gpsimd.memset(eps[:], 1e-12)

            xc = []
            for i in range(NK):
                t_ = big.tile([128, T], BF16, tag="big")
                nc.sync.dma_start(t_[:], xT_d[128 * i : 128 * (i + 1), :])
                xc.append(t_)

            # ---- projections: transposed layout [EPC, T], bf16 out ----
            def project(w_s):
                raw = prawp.tile([EPC, T], BF16, tag="praw")
                for n in range(NT):
                    ps = mmp.tile([128, 512], F32, tag="mm")
                    for i in range(NK):
                        nc.tensor.matmul(
                            ps[:],
                            w_s[:, 128 * i : 128 * (i + 1)],
                            xc[i][:, 512 * n : 512 * (n + 1)],
                            start=(i == 0),
                            stop=(i == NK - 1),
                        )
                    nc.vector.tensor_copy(raw[:, 512 * n : 512 * (n + 1)], ps[:])
                return raw

            vT_raw = project(wv_s)
            q_raw = project(wq_s)
            k_raw = project(wk_s)

            # ---- cosine-norm + scale + rotary (per 512-chunk) ----
            # rsqrt(sumsq) = exp(-0.5 * ln(sumsq)): the sumsq matmul lhsT is
            # hselw[e,d] = (head(e)==head(d)) which both reduces and
            # broadcasts across each head's 64 rows in one shot. Ln and Exp
            # are batched (all Lns, then all Exps) so ACT loads each table
            # once per tensor instead of per chunk.
            def norm_rot_both(raws):
                rots = [
                    big.tile([EPC, T], BF16, name=f"rot_{ti}", tag="big")
                    for ti in range(2)
                ]
                lws, rws = {}, {}
                for ti, raw in enumerate(raws):
                    for n in range(NT):
                        sl = slice(512 * n, 512 * (n + 1))
                        sq = smp.tile([128, 512], BF16, name=f"sq_{ti}_{n}", tag="sq")
                        nc.vector.tensor_mul(sq[:], raw[:, sl], raw[:, sl])
                        ssb = mmp.tile([128, 512], F32, name=f"ssb_{ti}_{n}", tag="mm")
                        nc.tensor.matmul(ssb[:], hselw[:], sq[:], start=True, stop=True)
                        lw = smp.tile(
                            [128, 512], F32, name=f"lw_{ti}_{n}", tag="lw", bufs=8
                        )
                        nc.scalar.activation(
                            lw[:], ssb[:], mybir.ActivationFunctionType.Ln, bias=eps[:]
                        )
                        lws[(ti, n)] = lw
                for ti in range(2):
                    for n in range(NT):
                        rw = smp.tile(
                            [128, 512], BF16, name=f"rw_{ti}_{n}", tag="rw", bufs=8
                        )
                        nc.scalar.activation(
                            rw[:], lws[(ti, n)][:],
                            mybir.ActivationFunctionType.Exp, scale=-0.5,
                        )
                        rws[(ti, n)] = rw
                # chunk-major so attention's first tq chunk unblocks earliest
                for n in range(NT):
                    for ti, raw in enumerate(raws):
                        sl = slice(512 * n, 512 * (n + 1))
                        qn = pnp.tile([128, 512], BF16, name=f"qn_{ti}_{n}", tag="pn")
                        nc.vector.tensor_mul(qn[:], raw[:, sl], rws[(ti, n)][:])
                        sw = pswp.tile([128, 512], BF16, name=f"sw_{ti}_{n}", tag="psw")
                        for (a, b) in ((0, 32), (32, 0), (64, 96), (96, 64)):
                            nc.gpsimd.dma_start(sw[a : a + 32, :], qn[b : b + 32, :])
                        nc.vector.tensor_mul(rots[ti][:, sl], qn[:], rota[:, sl])
                        nc.vector.tensor_mul(sw[:], sw[:], rotb[:, sl])
                        nc.vector.tensor_add(rots[ti][:, sl], rots[ti][:, sl], sw[:])
                return rots

            # ---- v to natural layout via hardware DMA transpose (bf16) ----
            # per 128-chunk j the layout is [v_h0 (64) | ones (64) | v_h1 (64)
            # | ones (64)], so each AV matmul lhsT is [128, 128] and writes
            # y_h to PSUM rows 0-63 and the softmax denominator Z_h
            # (pre-broadcast 64x) to rows 64-127.
            vext = wt.tile([128, (T // 128) * 256], BF16, tag="vext")
            nc.gpsimd.memset(vext[:], 1.0)
            ident = wt.tile([128, 128], BF16, tag="ident")
            nc.gpsimd.memset(ident[:], 0.0)
            nc.gpsimd.affine_select(
                out=ident[:],
                in_=ident[:],
                compare_op=mybir.AluOpType.not_equal,
                fill=1.0,
                base=0,
                pattern=[[-1, 128]],
                channel_multiplier=1,
            )
            vv = vext[:].rearrange("p (j h f) -> p j h f", h=HPC, f=128)
            for j in range(T // 128):
                tp_ = psp.tile([128, 128], BF16, tag="ps", name=f"vtp_{j}")
                nc.tensor.transpose(
                    tp_[:], vT_raw[:, 128 * j : 128 * (j + 1)], ident[:]
                )
                # rows->[ts], cols [h0 d | h1 d]; scatter the two head halves
                # into the [v | ones] slots of vext with one strided copy
                nc.vector.tensor_copy(
                    vv[:, j, :, 0:64],
                    tp_[:].rearrange("p (h f) -> p h f", h=HPC),
                )


            qrot, krot = norm_rot_both([q_raw, k_raw])

            # ---- attention (S^T layout) + Wo partial ----
            for c in range(NT):
                nts = 4 * c + 4
                cq = slice(512 * c, 512 * (c + 1))
                pyh = [
                    pyp.tile([128, 512], F32, name=f"py0_{c}", tag="py0", bufs=1),
                    pyp.tile([128, 512], F32, name=f"py1_{c}", tag="py1", bufs=1),
                ]
                for j in range(nts):
                    m = j - 4 * c
                    for h in range(HPC):
                        hs = slice(64 * h, 64 * (h + 1))
                        ps = psp.tile([128, 512], F32, tag="ps")
                        nc.tensor.matmul(
                            ps[:],
                            krot[hs, 128 * j : 128 * (j + 1)],
                            qrot[hs, cq],
                            start=True,
                            stop=True,
                            tile_position=(64 * h, 0),
                        )
                        pt = atp.tile([128, 512], BF16, tag=f"pt{h}", bufs=3)
                        nc.scalar.activation(
                            pt[:], ps[:], mybir.ActivationFunctionType.Exp,
                            scale=ATTN_SCALE,
                        )
                        if m >= 0:
                            # causal: keep pt[x, y] only where y >= x + 128*m
                            nc.gpsimd.affine_select(
                                out=pt[:],
                                in_=pt[:],
                                compare_op=mybir.AluOpType.is_ge,
                                fill=0.0,
                                base=-128 * m,
                                pattern=[[1, 512]],
                                channel_multiplier=-1,
                            )
                        nc.tensor.matmul(
                            pyh[h][:],
                            vext[:, 256 * j + 128 * h : 256 * j + 128 * (h + 1)],
                            pt[:],
                            start=(j == 0),
                            stop=(j == nts - 1),
                        )
                # normalize: yt rows of head h = py_h[0:64] * (1/Z_h);
                # Z_h sits pre-broadcast in py_h rows 64-127
                # copy y and Z out of PSUM immediately so the banks free for
                # the next chunk's AV; reciprocal + normalize run per 128-col
                # sub-chunk so the Wo matmuls pipeline behind them
                yt = atp.tile([128, 512], BF16, tag="yt", bufs=2)
                ysb, zs = [], []
                for h in range(HPC):
                    y_ = smp.tile([64, 512], F32, name=f"ysb_{c}_{h}", tag="ysb")
                    nc.vector.tensor_copy(y_[:], pyh[h][0:64, :])
                    z_ = smp.tile([64, 512], F32, name=f"zs_{c}_{h}", tag="zs")
                    nc.vector.tensor_copy(z_[:], pyh[h][64:128, :])
                    ysb.append(y_)
                    zs.append(z_)
                for mi in range(4):
                    ms = slice(128 * mi, 128 * (mi + 1))
                    for h in range(HPC):
                        hs = slice(64 * h, 64 * (h + 1))
                        zri = smp.tile(
                            [64, 128], F32, name=f"zri_{c}_{h}_{mi}", tag="zri"
                        )
                        nc.vector.reciprocal(zri[:], zs[h][:, ms])
                        nc.vector.tensor_mul(yt[hs, ms], ysb[h][:, ms], zri[:])
                    ost = atp.tile([128, D], F32, name=f"ost_{c}_{mi}", tag="ost", bufs=2)
                    r0 = 512 * c + 128 * mi
                    for nn in range(2):
                        po = mmp.tile([128, 512], F32, tag="mm")
                        nc.tensor.matmul(
                            po[:],
                            yt[:, ms],
                            wo_s[:, 512 * nn : 512 * (nn + 1)],
                            start=True,
                            stop=True,
                        )
                        nc.vector.tensor_copy(ost[:, 512 * nn : 512 * (nn + 1)], po[:])
                        nc.sync.dma_start(
                            out_d[r0 : r0 + 128, 512 * nn : 512 * (nn + 1)],
                            ost[:, 512 * nn : 512 * (nn + 1)],
                        )

    return nc


_NC = None
_NC_SPLIT = False


def _host_shards(x, Wq, Wk, Wv, Wo, s_qk):
    x = np.asarray(x, dtype=np.float32)
    Wq = np.asarray(Wq, dtype=np.float32)
    Wk = np.asarray(Wk, dtype=np.float32)
    Wv = np.asarray(Wv, dtype=np.float32)
    Wo = np.asarray(Wo, dtype=np.float32)
    s_qk = np.asarray(s_qk, dtype=np.float32)

    xT = np.ascontiguousarray(x.reshape(T, D).T).astype(NPBF16)

    dim_q = HD // 4
    freq = (1.0 / 1024.0) ** np.linspace(0.0, 1.0, dim_q, dtype=np.float32)
    freq = np.concatenate([freq, np.zeros(dim_q, np.float32)])
    theta = np.arange(T, dtype=np.float32)[:, None] * freq[None, :]
    cosT = np.cos(theta).T.astype(np.float32)
    sinT = np.sin(theta).T.astype(np.float32)
    A64 = np.concatenate([cosT, cosT], 0)
    B64 = np.concatenate([sinT, -sinT], 0)
    s_eff = s_qk * np.float32(np.sqrt(D))

    hselw = np.zeros((128, 128), np.float32)
    for h in range(HPC):
        hselw[64 * h : 64 * (h + 1), 64 * h : 64 * (h + 1)] = 1.0
    hselw = hselw.astype(NPBF16)

    in_maps = []
    for c in range(NCORES):
        cols = slice(EPC * c, EPC * (c + 1))
        rota_rows, rotb_rows = [], []
        for h in range(HPC):
            s = s_eff[HPC * c + h]
            s_swap = np.concatenate([s[32:], s[:32]])
            rota_rows.append(s[:, None] * A64)
            rotb_rows.append(s_swap[:, None] * B64)
        in_maps.append(
            {
                "xT": xT,
                "wq": np.ascontiguousarray(Wq[:, cols]).astype(NPBF16),
                "wk": np.ascontiguousarray(Wk[:, cols]).astype(NPBF16),
                "wv": np.ascontiguousarray(Wv[:, cols]).astype(NPBF16),
                "wo": np.ascontiguousarray(Wo[EPC * c : EPC * (c + 1), :]).astype(NPBF16),
                "rota": np.concatenate(rota_rows, 0).astype(NPBF16),
                "rotb": np.concatenate(rotb_rows, 0).astype(NPBF16),
                "hselw": hselw,
            }
        )
    return in_maps


def kernel(x, Wq, Wk, Wv, Wo, s_qk):
    global _NC, _NC_SPLIT
    _register_ntff_hook()
    if _NC is None:
        _NC = _build_nc()
    if not _NC_SPLIT:
        _split_ctrl_waits(_NC)
        _NC_SPLIT = True
    in_maps = _host_shards(x, Wq, Wk, Wv, Wo, s_qk)
    res = None
    last_exc = None
    for _attempt in range(3):
        try:
            res = run_bass_kernel_spmd(_NC, in_maps, list(range(NCORES)))
            break
        except Exception as e:  # transient NRT_EXEC_UNIT_UNRECOVERABLE seen on first runs
            last_exc = e
    if res is None:
        raise last_exc
    LAST["exec_time_ns"] = res.exec_time_ns
    LAST["trace"] = res.instructions_and_trace[1] if res.instructions_and_trace else None
    total = np.zeros((T, D), np.float64)
    for i in range(NCORES):
        total += res.results[i]["out"].astype(np.float64)
    return total.astype(np.float32).reshape(1, T, D)
